# revision 2
# baseline (speedup 1.0000x reference)
"""Bahdanau-additive attention scorer on 8 TRN2 NeuronCores — sine-separable.

reference:
  w = context @ Wc.T              (B, CTX, D)   "context projection"
  u = queries @ Wq.T + bq         (B, QRS, D)
  scores[b,c,q] = sum_e v[e] * tanh(w[b,c,e] + u[b,q,e])
  return scores.reshape(B, QRS, CTX)    # flat view of (B, CTX, QRS)

Key algorithmic move: tanh(w+u) ~= sum_k b_k sin(om_k (w+u)); the sine
addition formula makes every term exactly separable:
  sin(om(w+u)) = sin(om w)cos(om u) + cos(om w)sin(om u)
so scores ~= sum_k [ (v b_k sin(om_k w))^T-contract-cos(om_k u) + ... ]
— 4K small PE matmuls instead of a 134M-element tanh on ACT.

Sharding: (batch, ctx): core k handles batch k//4, ctx rows
(k%4)*256..+256, with all 256 queries of its batch; weights replicated.

Per-core device pipeline (d=hidden dim on partitions, 2 halves of 128):
  DMA  : inputs split across the sync + pool HW queues (parallel, ~1.4us
         vs 3.3us serialized); outputs on sync + scalar queues.
  PE   : 3 early dummy matmuls warm the HAM clock-gate; prologue
         wT = Wc@ctxT, uT = Wq@qT (fp16, psum fp32); 3 more dummies are
         pinned on w16 so the scheduler cannot hoist them in front of
         the prologue; main loop: out[ch] += F_feat.T @ G_feat.
  DVE  : cast wT/uT psum -> fp16 (bq folded into uT); range reduction
         for k=1..3: t0 = om/2pi*x + 1/8 (f16); t1 = f16(t0+1536) =
         1536+round(t0); r = t1-1536; p = t0-r in [-0.5,0.5] via
         tensor_scalar_add + tensor_tensor (4x/2x modes — no 1x-mode
         scalar_tensor_tensor); v*b_k fold into the F-side features.
  ACT  : Sin table load lands before the first activation (no explicit
         warm-up act needed); k=0 features straight from PSUM via the
         free affine (scale=om0, bias cols); 8 wrapped Sin instructions
         of [128, 3*256] (sin and cos via -/+ pi/4 bias, +1/8 offset in
         t0 cancels the bias); ch0 tail psum->sbuf copy runs on ACT so
         it overlaps the ch1 matmuls (which are ch-split in the last
         group for the same reason).

Output per core: out[256, 256] f32 = scores[b, cs + row, q]; host
reassembles and reshapes.

Validated: rel err 5.5e-3 (gate 2e-2); HW exec ~28.2-29.5us (vs 29.3-31.3
for the prior serial-DMA/STT version).
"""

import numpy as np

import concourse.bacc as bacc
import concourse.mybir as mybir
import concourse.tile as tile
from concourse.bass_utils import run_bass_kernel_spmd

F32 = mybir.dt.float32
F16 = mybir.dt.float16
SIN = mybir.ActivationFunctionType.Sin
MULT = mybir.AluOpType.mult
ADD = mybir.AluOpType.add
SUBTRACT = mybir.AluOpType.subtract
TWO_PI = float(2 * np.pi)

B, CTX, QRS, D = 2, 1024, 256, 256
N_CORES = 8
CSH = CTX // (N_CORES // B)      # 256 ctx rows per core
HALF_PI = float(np.pi / 2)

# sine decomposition of tanh(z), z = w+u (fit_sines.py; overwritten below
# by the validated constants once the fit completes)
# K=4 validated fit: end-to-end rel err 5.4e-3 incl fp16 (gate 2e-2).
# K=5 fallback: om=[0.321345, 0.971799, 1.637332, 2.396015, 3.480512],
#               b=[1.22307787, 0.30031635, 0.10450889, 0.04103795, 0.01116093]
OMEGAS = [0.323771, 0.972604, 1.686551, 2.755415]
COEFS = [1.22519756, 0.29400763, 0.12029881, 0.03474042]
K = len(OMEGAS)

DUMMY_PRE = 3
DUMMY_POST = 3


def _build_nc():
    nc = bacc.Bacc("TRN2", target_bir_lowering=False, debug=False,
                   enable_asserts=False)

    # inp cols: [WcT(256) | ctxT(256) | WqT(256) | qT(256)], rows = d_in
    inp_d = nc.dram_tensor("inp", [2 * 128, 1024], F16, kind="ExternalInput")
    # cst cols: [bq half0, bq half1 | v*b_k columns: col 2+2k+h]
    cst_d = nc.dram_tensor("cst", [128, 2 + 2 * K], F32, kind="ExternalInput")
    out_d = nc.dram_tensor("out", [2 * 128, QRS], F32, kind="ExternalOutput")

    with tile.TileContext(nc) as tc:
        with (
            tc.tile_pool(name="consts", bufs=1) as cp,
            tc.tile_pool(name="pp", bufs=1, space="PSUM") as pp,
        ):
            # ---------- input DMAs (sync queue; w-path first) ----------
            ih = [cp.tile([128, 1024], F16, tag=f"ih{h}", name=f"ih{h}")
                  for h in range(2)]
            cst = cp.tile([128, 2 + 2 * K], F32, tag="cst", name="cst")
            nc.sync.dma_start(ih[0][:, 0:512], inp_d[0:128, 0:512])
            nc.gpsimd.dma_start(ih[1][:, 0:512], inp_d[128:256, 0:512])
            nc.sync.dma_start(ih[0][:, 512:1024], inp_d[0:128, 512:1024])
            nc.gpsimd.dma_start(ih[1][:, 512:1024], inp_d[128:256, 512:1024])
            nc.sync.dma_start(cst[:], cst_d[:])

            # ---------- ACT table warmup (Sin), after scalar's DMA dispatches ----------
            warm = cp.tile([128, 1], F32, tag="warm", name="warm")
            hpi = cp.tile([128, 1], F32, tag="hpi", name="hpi")
            qpi_m = cp.tile([128, 1], F32, tag="qpim", name="qpi_m")
            qpi_p = cp.tile([128, 1], F32, tag="qpip", name="qpi_p")
            nc.vector.memset(warm[:], 0.0)
            nc.vector.memset(hpi[:], HALF_PI)
            nc.vector.memset(qpi_m[:], -HALF_PI / 2)
            nc.vector.memset(qpi_p[:], HALF_PI / 2)

            # ---------- PE warmup: dummy matmuls so HAM un-throttles ----------
            j1 = cp.tile([128, 128], F16, tag="j1", name="j1")
            j2 = cp.tile([128, 512], F16, tag="j2", name="j2")
            nc.vector.memset(j1[:], 0.0)
            nc.vector.memset(j2[:], 0.0)
            dps = pp.tile([128, 512], F32, tag="dps", name="dps")
            for i in range(DUMMY_PRE):
                nc.tensor.matmul(dps[:], lhsT=j1[:], rhs=j2[:],
                                 start=True, stop=True)

            # ---------- prologue: wT = Wc@ctxT, uT = Wq@qT ----------
            psw = [pp.tile([128, 512], F32, tag=f"psw{h}", name=f"psw{h}")
                   for h in range(2)]
            psu = [pp.tile([128, 512], F32, tag=f"psu{h}", name=f"psu{h}")
                   for h in range(2)]
            for h in range(2):
                for k in range(2):
                    nc.tensor.matmul(
                        psw[h][:, 0:256],
                        lhsT=ih[k][:, h * 128:(h + 1) * 128],
                        rhs=ih[k][:, 256:512],
                        start=k == 0, stop=k == 1)
            for h in range(2):
                for k in range(2):
                    nc.tensor.matmul(
                        psu[h][:, 0:256],
                        lhsT=ih[k][:, 512 + h * 128:512 + (h + 1) * 128],
                        rhs=ih[k][:, 768:1024],
                        start=k == 0, stop=k == 1)

            # ---------- k1 direct-feature bias columns ----------
            zcol = cp.tile([128, 1], F32, tag="zcol", name="zcol")
            nc.vector.memset(zcol[:], 0.0)
            bq1 = [[cp.tile([128, 1], F32, tag=f"bq1{h}{p}", name=f"bq1{h}{p}")
                    for p in range(2)] for h in range(2)]
            for h in range(2):
                nc.vector.tensor_scalar(bq1[h][0][:], cst[:, h:h + 1],
                                        float(OMEGAS[0]), 0.0, MULT, ADD)
                nc.vector.tensor_scalar(bq1[h][1][:], cst[:, h:h + 1],
                                        float(OMEGAS[0]), HALF_PI, MULT, ADD)

            # ---------- casts to fp16 (bq folded into u) ----------
            w16 = [cp.tile([128, 256], F16, tag=f"w16{h}", name=f"w16{h}")
                   for h in range(2)]
            u16 = [cp.tile([128, 256], F16, tag=f"u16{h}", name=f"u16{h}")
                   for h in range(2)]
            for h in range(2):
                nc.vector.tensor_copy(w16[h][:], psw[h][:, 0:256])
            for h in range(2):
                nc.vector.tensor_scalar_add(u16[h][:], psu[h][:, 0:256],
                                            cst[:, h:h + 1])
            for i in range(DUMMY_POST):
                nc.tensor.matmul(dps[:, 0:256], lhsT=w16[0][:, 0:128],
                                 rhs=w16[0][:], start=True, stop=True)

            # ---------- preps: range-reduced phase in cycles ----------
            # t0 = om/2pi * x + 1/8 (f16); t1 = f16(t0 + 1536) == 1536 +
            # round(t0) (f16 ulp at 1536 is 1.0); prep = (t0 + 1536) - t1
            # = t0 - round(t0) in [-0.5, 0.5] (fp32 internal keeps the
            # fraction).  ACT then evaluates sin(2pi*prep -/+ pi/4) =
            # sin/cos(om*x) exactly, args within the Sin table's range.
            t0F = [cp.tile([128, K * 256], F16, tag=f"t0F{h}", name=f"t0F{h}")
                   for h in range(2)]
            t0G = [cp.tile([128, K * 256], F16, tag=f"t0G{h}", name=f"t0G{h}")
                   for h in range(2)]
            t1F = [cp.tile([128, K * 256], F16, tag=f"t1F{h}", name=f"t1F{h}")
                   for h in range(2)]
            t1G = [cp.tile([128, K * 256], F16, tag=f"t1G{h}", name=f"t1G{h}")
                   for h in range(2)]
            rF = [cp.tile([128, K * 256], F16, tag=f"rF{h}", name=f"rF{h}")
                  for h in range(2)]
            rG = [cp.tile([128, K * 256], F16, tag=f"rG{h}", name=f"rG{h}")
                  for h in range(2)]
            pF = [cp.tile([128, K * 256], F16, tag=f"pF{h}", name=f"pF{h}")
                  for h in range(2)]
            pG = [cp.tile([128, K * 256], F16, tag=f"pG{h}", name=f"pG{h}")
                  for h in range(2)]
            WS = slice(256, K * 256)     # wrapped blocks k=1..K-1
            for h in range(2):
                for k in range(1, K):
                    nc.vector.tensor_scalar(
                        t0F[h][:, k * 256:(k + 1) * 256], w16[h][:],
                        float(OMEGAS[k] / (2 * np.pi)), 0.125, MULT, ADD)
                nc.vector.tensor_scalar_add(t1F[h][:, WS], t0F[h][:, WS],
                                            1536.0)
                nc.vector.tensor_scalar_add(rF[h][:, WS], t1F[h][:, WS],
                                            -1536.0)
                nc.vector.tensor_tensor(pF[h][:, WS], t0F[h][:, WS],
                                        rF[h][:, WS], SUBTRACT)
                for k in range(1, K):
                    nc.vector.tensor_scalar(
                        t0G[h][:, k * 256:(k + 1) * 256], u16[h][:],
                        float(OMEGAS[k] / (2 * np.pi)), 0.125, MULT, ADD)
                nc.vector.tensor_scalar_add(t1G[h][:, WS], t0G[h][:, WS],
                                            1536.0)
                nc.vector.tensor_scalar_add(rG[h][:, WS], t1G[h][:, WS],
                                            -1536.0)
                nc.vector.tensor_tensor(pG[h][:, WS], t0G[h][:, WS],
                                        rG[h][:, WS], SUBTRACT)

            # ---------- features (ACT Sin) + v*b fold + matmuls ----------
            sF = [cp.tile([128, K * 256], F16, tag=f"sF{h}", name=f"sF{h}")
                  for h in range(2)]
            cF = [cp.tile([128, K * 256], F16, tag=f"cF{h}", name=f"cF{h}")
                  for h in range(2)]
            sG = [cp.tile([128, K * 256], F16, tag=f"sG{h}", name=f"sG{h}")
                  for h in range(2)]
            cG = [cp.tile([128, K * 256], F16, tag=f"cG{h}", name=f"cG{h}")
                  for h in range(2)]
            vsF = [cp.tile([128, K * 256], F16, tag=f"vsF{h}", name=f"vsF{h}")
                   for h in range(2)]
            vcF = [cp.tile([128, K * 256], F16, tag=f"vcF{h}", name=f"vcF{h}")
                   for h in range(2)]
            out_ps = [pp.tile([128, 512], F32, tag=f"ops{c}", name=f"ops{c}")
                      for c in range(2)]

            def vmults(src, dst, h):
                for k in range(K):
                    nc.vector.tensor_scalar_mul(
                        dst[h][:, k * 256:(k + 1) * 256],
                        src[h][:, k * 256:(k + 1) * 256],
                        cst[:, 2 + 2 * k + h:3 + 2 * k + h])

            def mms(feat, g, h, first=False, last=False):
                if last:
                    for ch in range(2):
                        for k in range(K):
                            nc.tensor.matmul(
                                out_ps[ch][:, 0:256],
                                lhsT=feat[h][:, k * 256 + ch * 128:
                                             k * 256 + ch * 128 + 128],
                                rhs=g[h][:, k * 256:(k + 1) * 256],
                                start=False, stop=k == K - 1)
                    return
                for k in range(K):
                    for ch in range(2):
                        nc.tensor.matmul(
                            out_ps[ch][:, 0:256],
                            lhsT=feat[h][:, k * 256 + ch * 128:
                                         k * 256 + ch * 128 + 128],
                            rhs=g[h][:, k * 256:(k + 1) * 256],
                            start=first and k == 0,
                            stop=last and k == K - 1)

            OM1 = float(OMEGAS[0])
            for h in range(2):
                nc.scalar.activation(sF[h][:, 0:256], psw[h][:, 0:256], SIN,
                                     bias=zcol[:], scale=OM1)
                nc.scalar.activation(cF[h][:, 0:256], psw[h][:, 0:256], SIN,
                                     bias=hpi[:], scale=OM1)
                nc.scalar.activation(sG[h][:, 0:256], psu[h][:, 0:256], SIN,
                                     bias=bq1[h][0][:], scale=OM1)
                nc.scalar.activation(cG[h][:, 0:256], psu[h][:, 0:256], SIN,
                                     bias=bq1[h][1][:], scale=OM1)
            for h in range(2):
                nc.scalar.activation(sF[h][:, WS], pF[h][:, WS], SIN,
                                     bias=qpi_m[:], scale=TWO_PI)
                vmults(sF, vsF, h)
                nc.scalar.activation(cG[h][:, WS], pG[h][:, WS], SIN,
                                     bias=qpi_p[:], scale=TWO_PI)
                mms(vsF, cG, h, first=h == 0)
                for i in range(2):
                    nc.tensor.matmul(dps[:], lhsT=j1[:], rhs=j2[:],
                                     start=True, stop=True)
                nc.scalar.activation(cF[h][:, WS], pF[h][:, WS], SIN,
                                     bias=qpi_p[:], scale=TWO_PI)
                vmults(cF, vcF, h)
                nc.scalar.activation(sG[h][:, WS], pG[h][:, WS], SIN,
                                     bias=qpi_m[:], scale=TWO_PI)
                if h == 0:
                    for i in range(2):
                        nc.tensor.matmul(dps[:], lhsT=j1[:], rhs=j2[:],
                                         start=True, stop=True)
                mms(vcF, sG, h, last=h == 1)

            # ---------- tail: psum -> sbuf f32 -> DRAM ----------
            oS = [cp.tile([128, 256], F32, tag=f"oS{c}", name=f"oS{c}")
                  for c in range(2)]
            IDENT = mybir.ActivationFunctionType.Identity
            nc.scalar.activation(oS[0][:], out_ps[0][:, 0:256], IDENT,
                                 bias=zcol[:])
            nc.vector.tensor_copy(oS[1][:], out_ps[1][:, 0:256])
            nc.sync.dma_start(out_d[0:128, :], oS[0][:])
            nc.scalar.dma_start(out_d[128:256, :], oS[1][:])

    nc.compile()
    return nc


_NC_CACHE = {}


def _get_nc():
    if "nc" not in _NC_CACHE:
        _NC_CACHE["nc"] = _build_nc()
    return _NC_CACHE["nc"]


def _in_maps(context, queries, Wc, Wq, bq, v):
    f16, f32 = np.float16, np.float32
    wcT = np.ascontiguousarray(Wc.T.astype(f16))          # [d_in, e]
    wqT = np.ascontiguousarray(Wq.T.astype(f16))
    bq2 = bq.reshape(2, 128).T.astype(f32)                # [128, 2]
    vb = np.empty((128, 2 * K), dtype=f32)
    for k in range(K):
        for h in range(2):
            vb[:, 2 * k + h] = v[0, h * 128:(h + 1) * 128] * COEFS[k]
    cst = np.ascontiguousarray(np.concatenate([bq2, vb], axis=1))
    maps = []
    for core in range(N_CORES):
        b = core // (N_CORES // B)
        cs = (core % (N_CORES // B)) * CSH
        ctxT = np.ascontiguousarray(context[b, cs:cs + CSH, :].T.astype(f16))
        qT = np.ascontiguousarray(queries[b].T.astype(f16))
        inp = np.ascontiguousarray(
            np.concatenate([wcT, ctxT, wqT, qT], axis=1))
        maps.append({"inp": inp, "cst": cst})
    return maps


def run(context, queries, Wc, Wq, bq, v, trace=False, **spmd_kwargs):
    nc = _get_nc()
    maps = _in_maps(np.asarray(context), np.asarray(queries), np.asarray(Wc),
                    np.asarray(Wq), np.asarray(bq), np.asarray(v))
    res = run_bass_kernel_spmd(nc, maps, core_ids=list(range(N_CORES)),
                               trace=trace, **spmd_kwargs)
    scores = np.empty((B, CTX, QRS), dtype=np.float32)
    for core in range(N_CORES):
        b = core // (N_CORES // B)
        cs = (core % (N_CORES // B)) * CSH
        scores[b, cs:cs + CSH, :] = res.results[core]["out"]
    return scores.reshape(B, QRS, CTX), res


def kernel(context, queries, Wc, Wq, bq, v):
    out, _ = run(context, queries, Wc, Wq, bq, v, trace=False)
    return out

